# revision 4
# baseline (speedup 1.0000x reference)
"""Multi-head attention (B=2, S=2048, D=1024, H=16, hd=64) on 8 NeuronCores.

v4: v3 + PV issued as trailing back-to-back DoubleRow chains per
(head, qt) after the kb loop (chained accumulation pipelines at ~stream
rate; interleaved PV paid ~470ns/matmul vs ~230 chained).

Sharding: core c -> batch b=c//4, head-group hg=c%4 (4 heads each).

Changes vs v2 (267978ns baseline):
- exp split 3 ways by kb: ACT runs real Exp -> fp8; DVE and Pool compute
  u8 = s*(8/ln2) + C via tensor_scalar, written through a uint8 bitcast of
  the fp8 ex buffer (Schraudolph: the u8 bits ARE the fp8 e4m3 encoding of
  ~exp(s)). Kills the 1.04us/kb ACT serialization (was 157us busy).
- qkv bias matmuls gone: qk bias folded into the PSUM drain as a
  per-partition tensor_scalar add; v bias folded into bout on the host
  (softmax weights sum to 1, so +bv commutes with attention exactly).
- finish_head: denominator reciprocal on DVE (one op), broadcast matmul in
  f32r (no bf16 cast), multiply reads rps PSUM directly (no recs copy).
- out-projection drains on gpsimd; outputs written bf16 (half the DMA).
Host gathers: sum head-group partials per batch, transpose, add bout_eff.
"""
import sys
import types

import numpy as np
from contextlib import ExitStack

D = 1024
S = 2048
B = 2
HPC = 4          # heads per core
HD = 64          # head dim
NCORES = 8
QT = 512         # query tile
NQT = S // QT    # 4
KB = 128         # key block
NKB = S // KB    # 16
VW = HD + 1      # v width incl. ones column = 65
VROW = HPC * HD  # 256 (no ones cols in wv anymore)

# exp engine map per kb: A=ACT exact exp, D=DVE u8 trick (gpsimd cannot
# read PSUM, so Pool is out of the exp business)
EXMAP = "ADADADADADADADAD"
C1 = 8.0 / np.log(2.0)
C2_ROUND = 56.0 - 0.345
C2_FLOOR = 56.5 - 0.345
C2 = C2_ROUND   # set per probe result

_CACHE = {}


def _split_sync_waits(bir):
    """Walrus CoreV2/V3 codegen rejects >1 sync wait on one instruction
    ('Too many sync wait commands'). Hoist excess waits onto ENGINE_NOPs
    injected just before the offender in the same engine stream."""
    n = 0
    for fn in bir["functions"]:
        for blk in fn["blocks"]:
            out = []
            for inst in blk["instructions"]:
                si = inst.get("sync_info")
                ow = (si or {}).get("on_wait") or []
                if si is not None and len(ow) > 1:
                    for w in ow[:-1]:
                        n += 1
                        out.append({
                            "debug": inst.get("debug", 0),
                            "engine": inst["engine"],
                            "ins": [],
                            "name": f"I-ws{n}",
                            "opcode": "EventSemaphore",
                            "outs": [],
                            "sync_info": {"on_wait": [w], "on_update": []},
                        })
                    si["on_wait"] = [ow[-1]]
                out.append(inst)
            blk["instructions"] = out
    return bir


def _install_support():
    import json

    import concourse.bass as bass_mod
    import concourse.tile as tile_mod

    if not getattr(bass_mod.Bass, "_waitsplit_patched", False):
        orig = bass_mod.Bass.to_json_bytes

        def to_json_bytes(self, *a, **kw):
            data = json.loads(orig(self, *a, **kw))
            _split_sync_waits(data)
            return json.dumps(data).encode()

        bass_mod.Bass.to_json_bytes = to_json_bytes
        bass_mod.Bass._waitsplit_patched = True
    if not getattr(tile_mod.TileContext, "_drain_patched", False):
        import bass_rust
        ScopedClock = tile_mod.ScopedClock

        def _drain_and_barrier(self, tick_clock, wait_clock):
            drain_inst = self.nc.sync.drain()
            wait_clock.add_sem_waits(
                drain_inst.ins, ScopedClock({None: tick_clock.global_clock})
            )
            si = drain_inst.ins.sync_info
            if si is not None and len(si.on_wait) > 1:
                waits = list(si.on_wait)
                drain_inst.ins.sync_info = bass_rust.SyncInfo(
                    on_wait=waits[:1], on_update=list(si.on_update)
                )
                for w in waits[1:]:
                    extra = self.nc.sync.drain()
                    extra.ins.sync_info = bass_rust.SyncInfo(on_wait=[w], on_update=[])
            self.nc.all_engine_barrier()
            assert self.sems is not None
            popped = self.nc._tile_sem_poison_stack.pop()
            assert popped is self._sem_poison
            self.nc.clear_and_free_semaphores(list(self.sems.allocated().values()))
            self.nc.all_engine_barrier()

        tile_mod.TileContext._drain_and_barrier = _drain_and_barrier
        tile_mod.TileContext._drain_patched = True

    try:
        import antenv
        if "antenv.axon_hooks" not in sys.modules:
            mod = types.ModuleType("antenv.axon_hooks")
            mod._hook = None

            def set_axon_ntff_profile_hook(h, _mod=mod):
                _mod._hook = h

            def get_axon_ntff_profile_hook(_mod=mod):
                return _mod._hook

            mod.set_axon_ntff_profile_hook = set_axon_ntff_profile_hook
            mod.get_axon_ntff_profile_hook = get_axon_ntff_profile_hook
            sys.modules["antenv.axon_hooks"] = mod
            antenv.axon_hooks = mod
        from trn_agent_boot.trn_boot import _ntff_profile_via_ctypes
        hook = _ntff_profile_via_ctypes("/opt/axon/libaxon_pjrt.so")
        sys.modules["antenv.axon_hooks"].set_axon_ntff_profile_hook(hook)
        import concourse.bass_utils as bass_utils
        bass_utils.upload_artifacts = lambda d: d
    except Exception:
        pass


def _build_nc():
    import concourse.bass as bass
    import concourse.tile as tile
    from concourse import mybir

    f32 = mybir.dt.float32
    bf16 = mybir.dt.bfloat16
    u8 = mybir.dt.uint8
    AF = mybir.ActivationFunctionType
    OP = mybir.AluOpType

    fp8 = mybir.dt.float8e4
    DR = mybir.MatmulPerfMode.DoubleRow

    nc = bass.Bass("TRN2", target_bir_lowering=False, debug=False,
                   num_devices=NCORES)
    xT_d = nc.dram_tensor("xT", [D, S], bf16, kind="ExternalInput").ap()
    wqk_d = nc.dram_tensor("wqk", [D, 512], bf16, kind="ExternalInput").ap()
    bqk_d = nc.dram_tensor("bqk", [128, 4], f32, kind="ExternalInput").ap()
    wv_d = nc.dram_tensor("wv", [D, VROW], bf16, kind="ExternalInput").ap()
    wo_d = nc.dram_tensor("wo", [256, D], bf16, kind="ExternalInput").ap()
    out_d = nc.dram_tensor("out", [D, S], bf16, kind="ExternalOutput").ap()

    with tile.TileContext(nc) as tc, ExitStack() as ctx:
        persist = ctx.enter_context(tc.tile_pool(name="persist", bufs=1))
        # PSUM: ss 2x[128,1024] = 4 banks; pvE+pvO = 2; proj 1; rps 1.
        ssp = ctx.enter_context(
            tc.tile_pool(name="ssp", bufs=2, space=bass.MemorySpace.PSUM))
        pvE = ctx.enter_context(
            tc.tile_pool(name="pvE", bufs=1, space=bass.MemorySpace.PSUM))
        pvO = ctx.enter_context(
            tc.tile_pool(name="pvO", bufs=1, space=bass.MemorySpace.PSUM))
        projp = ctx.enter_context(
            tc.tile_pool(name="projp", bufs=1, space=bass.MemorySpace.PSUM))
        rpsp = ctx.enter_context(
            tc.tile_pool(name="rpsp", bufs=1, space=bass.MemorySpace.PSUM))
        denp = ctx.enter_context(tc.tile_pool(name="denp", bufs=6))
        otmp = ctx.enter_context(tc.tile_pool(name="otmp", bufs=2))
        outp = ctx.enter_context(tc.tile_pool(name="outp", bufs=3))

        xT_sb = persist.tile([128, 8 * S], bf16)
        wqk_sb = persist.tile([128, 8 * 512], bf16)
        bqk_sb = persist.tile([128, 4], f32)
        wv_sb = persist.tile([128, 8 * VROW], bf16)
        wo_sb = persist.tile([128, 2 * D], bf16)
        qkT_sb = persist.tile([128, 4 * S], bf16)
        # v in fp8, pair-block layout for DoubleRow PV (t-slabs padded to 80
        # cols: walrus requires the k-tile-pair stride to be 16B aligned):
        # col = pair*640 + head*160 + (kb%2)*80 + dim ; dim 64 = ones
        v_sb = persist.tile([128, 8 * 640], fp8)
        # exp scores in fp8: per kb block of 1024 = [even-head 512 | odd 512]
        ex_sb = persist.tile([128, NKB * 1024], fp8)
        outT_sb = persist.tile([128, 2 * S], bf16)
        ones_sb = persist.tile([1, 512], bf16)
        nc.vector.memset(ones_sb[:], 1.0)
        ones64 = persist.tile([1, 64], bf16)
        nc.vector.memset(ones64[:], 1.0)
        # ones columns of v: dim 64 of each (pair, head, half) slab
        for pr in range(8):
            nc.gpsimd.memset(
                v_sb[:, pr * 640:(pr + 1) * 640]
                .rearrange("p (h f d) -> p h f d", h=4, f=2)[:, :, :, 64:65],
                1.0)
        # scratch for warm-up dummies: K=128 so the whole PE array shows
        # activity to the HAM clock gate (K=1 dummies don't register)
        z128 = persist.tile([128, 512], bf16)
        nc.vector.memset(z128[:], 0.0)

        # Input DMA: wqk + xT kb-major first (the prefix consumes them in
        # kb order) over 4 queues; wv/wo/bqk after (first needed much later).
        qs = [nc.sync, nc.scalar, nc.gpsimd]
        qi = 0
        for kb in range(8):
            qs[qi % 3].dma_start(wqk_sb[:, kb * 512:(kb + 1) * 512],
                                 wqk_d[kb * 128:(kb + 1) * 128, :])
            qi += 1
            for h2 in range(2):
                qs[qi % 3].dma_start(
                    xT_sb[:, kb * S + h2 * 1024: kb * S + (h2 + 1) * 1024],
                    xT_d[kb * 128:(kb + 1) * 128, h2 * 1024:(h2 + 1) * 1024])
                qi += 1
        nc.sync.dma_start(bqk_sb[:], bqk_d[:])
        for kb in range(8):
            qs[(qi + kb) % 3].dma_start(wv_sb[:, kb * VROW:(kb + 1) * VROW],
                                        wv_d[kb * 128:(kb + 1) * 128, :])
        for cb in range(2):
            nc.gpsimd.dma_start(wo_sb[:, cb * D:(cb + 1) * D],
                                wo_d[cb * 128:(cb + 1) * 128, :])

        # ---- projection emitters (PE matmuls + drains), run as filler
        # m-groups in wqk/qkT: m0=q(h0|h1) m1=q(h2|h3) m2=k(h0|h1) m3=k(h2|h3)
        def drain_qk(m, nt, ps):
            nc.vector.tensor_scalar(
                qkT_sb[:, m * S + nt * QT: m * S + (nt + 1) * QT], ps,
                bqk_sb[:, m:m + 1], None, OP.add)

        def v_tile_gen(sb):
            ps = projp.tile([128, VROW], f32, tag="proj")
            for kb in range(4):
                nc.tensor.matmul(
                    ps[:],
                    xT_sb[:, kb * S + sb * 128: kb * S + (sb + 1) * 128],
                    wv_sb[:, kb * VROW:(kb + 1) * VROW],
                    start=(kb == 0), stop=False, skip_group_check=True)
            yield 1040
            for kb in range(4, 8):
                nc.tensor.matmul(
                    ps[:],
                    xT_sb[:, kb * S + sb * 128: kb * S + (sb + 1) * 128],
                    wv_sb[:, kb * VROW:(kb + 1) * VROW],
                    start=False, stop=(kb == 7), skip_group_check=True)
            # drain to fp8 pair-block layout, one strided cast [128, 4, 64]
            vdst = (v_sb[:, :]
                    .rearrange("p (r h f d) -> p r h f d", r=8, h=4, f=2)
                    [:, sb // 2, :, sb % 2, 0:HD])
            nc.scalar.activation(
                vdst, ps[:].rearrange("p (h d) -> p h d", h=4), AF.Copy)
            yield 1560

        vstate = {"active": None, "done": 0}
        vq = []

        def advance_v(budget, min_done):
            spent = 0
            while vstate["active"] is not None or vq:
                if vstate["done"] >= min_done and spent >= budget:
                    return
                if vstate["active"] is None:
                    vq_gen = vq.pop(0)
                    vstate["active"] = vq_gen()
                try:
                    spent += next(vstate["active"])
                except StopIteration:
                    vstate["active"] = None
                    vstate["done"] += 1

        # filler queue of generator factories; each generator emits one proj
        # tile in ~512-1536 cycle granules (yielding its granule cost) so
        # filler never delays the next scores pack by more than ~0.5us in
        # the in-order PE stream. projp has bufs=1: only the single active
        # generator owns a proj psum tile at a time (rps uses its own pool),
        # so the accumulation isn't clobbered between granules.
        def qk_tile_gen(m, nt):
            ps = projp.tile([128, QT], f32, tag="proj")
            for kb in range(8):
                nc.tensor.matmul(
                    ps[:],
                    wqk_sb[:, kb * 512 + m * 128: kb * 512 + (m + 1) * 128],
                    xT_sb[:, kb * S + nt * QT: kb * S + (nt + 1) * QT],
                    start=(kb == 0), stop=(kb == 7), skip_group_check=True)
                if kb in (1, 3, 5):
                    yield 1024
            drain_qk(m, nt, ps[:])
            yield 1536

        def out_tile_gen(qt, dt):
            # alternate psum pools so consecutive out tiles pipeline (the
            # tail's 8 tiles otherwise serialize on a single bank)
            pool = projp if dt % 2 == 0 else rpsp
            ps = pool.tile([128, QT], f32, tag="proj" if dt % 2 == 0 else "rps")
            for cb in range(2):
                nc.tensor.matmul(
                    ps[:],
                    wo_sb[:, cb * D + dt * 128: cb * D + (dt + 1) * 128],
                    outT_sb[:, cb * S + qt * QT: cb * S + (qt + 1) * QT],
                    start=(cb == 0), stop=(cb == 1), skip_group_check=True)
            osb = outp.tile([128, QT], bf16, tag="ob")
            nc.scalar.activation(osb[:], ps[:], AF.Copy)
            nc.sync.dma_start(
                out_d[dt * 128:(dt + 1) * 128, qt * QT:(qt + 1) * QT], osb[:])
            yield 1024

        filler = []
        for nt in range(2, 4):
            filler.append(lambda nt=nt: qk_tile_gen(0, nt))
        for nt in range(4):
            filler.append(lambda nt=nt: qk_tile_gen(3, nt))
        for nt in range(4):
            filler.append(lambda nt=nt: qk_tile_gen(1, nt))

        fstate = {"active": None}

        def pop_filler(budget, pad=False):
            spent = 0
            while spent < budget:
                if fstate["active"] is None:
                    if not filler:
                        if pad:
                            # keep the PE array active for the HAM clock
                            # gate when there's no real filler left
                            for _ in range(4):
                                nc.tensor.ldweights(weights=z128[:, 0:128])
                        return
                    fstate["active"] = filler.pop(0)()
                try:
                    spent += next(fstate["active"])
                except StopIteration:
                    fstate["active"] = None

        # ---- prefix: k(h0|h1) all 4 tiles + q(h0|h1) qt0-1, kb-major so PE
        # consumes xT chunks as the DMAs land. Dummy matmuls (zeros, no DMA
        # deps) pad the PE stream so the HAM clock gate warms up during the
        # DMA wait instead of running the prefix at 1.2 GHz.
        ssA = ssp.tile([128, 2 * QT], f32, tag="s")
        ssB = ssp.tile([128, 2 * QT], f32, tag="s")
        pjA = projp.tile([128, QT], f32, tag="proj")
        pjB = rpsp.tile([128, QT], f32, tag="rps")
        dum = pvE.tile([128, QT], f32, tag="pvA")

        def dummy_mm(n):
            for _ in range(n):
                nc.tensor.matmul(
                    dum[:], z128[:, 0:128], z128[:],
                    start=True, stop=True, skip_group_check=True)

        dummy_mm(10)
        # slot -> (psum ap, m, nt)
        pre = [(ssA[:, 0:QT], 2, 0), (ssA[:, QT:2 * QT], 2, 1),
               (pjA[:], 0, 0), (pjB[:], 0, 1),
               (ssB[:, 0:QT], 2, 2), (ssB[:, QT:2 * QT], 2, 3)]
        for kb in range(8):
            for ap, m, nt in pre:
                nc.tensor.matmul(
                    ap,
                    wqk_sb[:, kb * 512 + m * 128: kb * 512 + (m + 1) * 128],
                    xT_sb[:, kb * S + nt * QT: kb * S + (nt + 1) * QT],
                    start=(kb == 0), stop=(kb == 7), skip_group_check=True)
            dummy_mm(2)
        for ap, m, nt in pre:
            drain_qk(m, nt, ap)
        for sb in range(NKB):
            vq.append(lambda sb=sb: v_tile_gen(sb))
        advance_v(0, 2)

        # ---- attention: units (hp, qt), hp-major
        def emit_exp(kb, ss):
            eng = EXMAP[kb % 16]
            dst = ex_sb[:, kb * 1024:(kb + 1) * 1024]
            if eng == "A":
                nc.scalar.activation(dst, ss[:], AF.Exp)
            elif eng == "D":
                nc.vector.tensor_scalar(
                    dst.bitcast(u8), ss[:], C1, C2, OP.mult, OP.add)
            else:
                nc.gpsimd.tensor_scalar(
                    dst.bitcast(u8), ss[:], C1, C2, OP.mult, OP.add)

        def finish_head(pv, h, qt):
            # 1/denominator via ACT ln -> exp(-x) (InstReciprocal on DVE is
            # 3.3us/tile -- measured; Ln+Exp is 2x0.6us and ACT has slack)
            rln = denp.tile([1, QT], f32, tag="rln")
            nc.scalar.activation(rln[:], pv[64:65, :], AF.Ln)
            recb = denp.tile([1, QT], bf16, tag="recb")
            nc.scalar.activation(recb[:], rln[:], AF.Exp, scale=-1.0)
            rps = rpsp.tile([64, QT], f32, tag="rps")
            nc.tensor.matmul(rps[:], ones64[:], recb[:],
                             start=True, stop=True, skip_group_check=True)
            # tensor_tensor may read only one PSUM operand: recs to SBUF
            # first (bf16 is lossless here, recb was already bf16)
            recs = denp.tile([64, QT], bf16, tag="recs")
            nc.vector.tensor_copy(recs[:], rps[:])
            cb = h // 2
            ocol = cb * S + qt * QT
            if h % 2 == 0:
                nc.vector.tensor_tensor(
                    outT_sb[0:64, ocol: ocol + QT], pv[0:64, :], recs[:],
                    OP.mult)
            else:
                ot = otmp.tile([64, QT], bf16, tag="o")
                nc.vector.tensor_tensor(ot[:], pv[0:64, :], recs[:], OP.mult)
                nc.gpsimd.dma_start(outT_sb[64:128, ocol: ocol + QT], ot[:])

        pend = []
        for hp in range(2):          # head pair: heads (2hp, 2hp+1)
            qm = hp                  # q m-group
            km = 2 + hp              # k m-group
            he = 2 * hp * 160        # v col offset of even head (pair layout)
            ho = (2 * hp + 1) * 160
            for qt in range(4):
                pe = pvE.tile([VW, QT], f32, tag="pvA")
                po = pvO.tile([VW, QT], f32, tag="pvB")
                for kb in range(NKB):
                    if kb in (1, 3) and pend:
                        pend.pop(0)()
                    ss = ssp.tile([128, 2 * QT], f32, tag="s")
                    klo = km * S + kb * 128
                    qlo = qm * S + qt * QT
                    nc.tensor.matmul(
                        ss[:, 0:QT], qkT_sb[0:64, klo: klo + 128],
                        qkT_sb[0:64, qlo: qlo + QT],
                        start=True, stop=True, skip_group_check=True,
                        tile_position=(0, 0))
                    nc.tensor.matmul(
                        ss[:, QT:2 * QT], qkT_sb[64:128, klo: klo + 128],
                        qkT_sb[64:128, qlo: qlo + QT],
                        start=True, stop=True, skip_group_check=True,
                        tile_position=(64, 0))
                    emit_exp(kb, ss)
                    # unit 1: pace vproj generators; later units: filler on
                    # even iters only so the exp pipeline stays the gate
                    if hp == 0 and qt == 0:
                        advance_v(1400, 0)
                    elif kb % 2 == 0:
                        pop_filler(2048)
                if hp == 0 and qt == 0:
                    advance_v(1 << 30, NKB)
                # trailing PV: per head a back-to-back chained DR accumulation
                # over kb-pairs. The last two pairs read the freshest exps
                # (still in flight), so they are deferred into the next
                # unit's kb loop together with the finish.
                def pv_part(pv, hoff, half, prs, start, stop):
                    for pr in prs:
                        vbase = pr * 640
                        vl = v_sb[:, vbase + hoff: vbase + hoff + 160]
                        vl = vl.rearrange("p (t m) -> p t m", t=2)[:, :, 0:VW]
                        exr = ex_sb[:, pr * 2048:(pr + 1) * 2048]
                        exr = exr.rearrange("p (t c) -> p t c", t=2)
                        nc.tensor.matmul(
                            pv[:], vl,
                            exr[:, :, half * QT:(half + 1) * QT],
                            start=(start and pr == prs[0]),
                            stop=(stop and pr == prs[-1]),
                            perf_mode=DR, skip_group_check=True)

                for pv, hoff, half in ((pe, he, 0), (po, ho, 1)):
                    pv_part(pv, hoff, half, list(range(6)), True, False)
                # defer PV tail + finish (ACT Ln/Exp round-trip) into the
                # next unit's kb loop so the PE keeps streaming meanwhile
                def _pv_tail(pe=pe, po=po, he=he, ho=ho):
                    pv_part(pe, he, 0, [6, 7], False, True)
                    pv_part(po, ho, 1, [6, 7], False, True)
                def _fin(pe=pe, po=po, hp=hp, qt=qt):
                    finish_head(pe, 2 * hp, qt)
                    finish_head(po, 2 * hp + 1, qt)
                    if hp == 1:
                        for dt in range(8):
                            filler.append(
                                lambda qt=qt, dt=dt: out_tile_gen(qt, dt))
                pend.append(_pv_tail)
                pend.append(_fin)
        for f in pend:
            f()
        pend.clear()
        # tail: remaining out projection (at least qt3)
        pop_filler(1 << 30)
    return nc


def _get_nc():
    if "nc" not in _CACHE:
        _install_support()
        _CACHE["nc"] = _build_nc()
    return _CACHE["nc"]


LAST_EXEC_NS = None


def kernel(x, Wqkv, bqkv, Wout, bout):
    from ml_dtypes import bfloat16
    from concourse.bass_utils import run_bass_kernel_spmd

    nc = _get_nc()

    x = np.asarray(x, np.float32)
    Wqkv = np.asarray(Wqkv, np.float32)
    bqkv = np.asarray(bqkv, np.float32)
    Wout = np.asarray(Wout, np.float32)
    bout = np.asarray(bout, np.float32)

    xT = [np.ascontiguousarray(x[b].T).astype(bfloat16) for b in range(B)]

    # v bias folds into the output bias exactly (softmax weights sum to 1)
    bv_full = np.concatenate(
        [bqkv[h * 192 + 128: h * 192 + 192] for h in range(16)])
    bout_eff = bout + bv_full @ Wout

    in_maps = []
    for c in range(NCORES):
        b, hg = divmod(c, HPC)
        heads = [hg * HPC + j for j in range(HPC)]

        wqk = np.empty((D, 512), np.float32)
        bqk = np.empty(512, np.float32)
        for j, h in enumerate(heads):
            base = h * 192
            wqk[:, j * 64:(j + 1) * 64] = Wqkv[:, base:base + 64] * 0.125
            wqk[:, 256 + j * 64: 256 + (j + 1) * 64] = Wqkv[:, base + 64:base + 128]
            bqk[j * 64:(j + 1) * 64] = bqkv[base:base + 64] * 0.125
            bqk[256 + j * 64: 256 + (j + 1) * 64] = bqkv[base + 64:base + 128]
        # per-partition bias layout [128, 4]: col m, partition p -> bias of
        # wqk column m*128+p
        bqk_cols = np.ascontiguousarray(bqk.reshape(4, 128).T)

        wv = np.empty((D, VROW), np.float32)
        for j, h in enumerate(heads):
            base = h * 192 + 128
            wv[:, j * HD:(j + 1) * HD] = Wqkv[:, base:base + 64]

        wo = np.ascontiguousarray(Wout[hg * 256:(hg + 1) * 256, :])

        in_maps.append({
            "xT": xT[b],
            "wqk": wqk.astype(bfloat16),
            "bqk": bqk_cols.astype(np.float32),
            "wv": wv.astype(bfloat16),
            "wo": wo.astype(bfloat16),
        })

    res = run_bass_kernel_spmd(nc, in_maps, core_ids=list(range(NCORES)))
    global LAST_EXEC_NS
    LAST_EXEC_NS = getattr(res, "exec_time_ns", None)

    out = np.empty((B, S, D), np.float32)
    for b in range(B):
        acc = res.results[b * HPC + 0]["out"].astype(np.float32)
        for hg in range(1, HPC):
            acc = acc + res.results[b * HPC + hg]["out"].astype(np.float32)
        out[b] = acc.T + bout_eff
    return out
